# revision 27
# baseline (speedup 1.0000x reference)
"""Trainium2 Bass kernel for BiaffineSpanHead.

Math (per batch b):
  Hs = x @ Ws_w.T + Ws_b            [S, H]
  He = x @ We_w.T + We_b            [S, H]
  biaff[s,e,c] = sum_{h,g} Hs[s,h] U[h,c,g] He[e,g]
  out[s,e,c] = biaff + Ls[s,c] + Le[e,c] + W_b[c]
      Ls = Hs @ W_w[:, :H].T,  Le = He @ W_w[:, H:].T

Sharding: data-parallel over batch B=8 across 8 cores (one batch each).
Per-core device layout: out[c, s, e] (contiguous [32, 512, 512] f32);
host transposes back to [B, S, E, C].

Device dataflow per core (P=128 partitions):
  xT [D,S] -> 6 SBUF tiles [128, 512] bf16
  HsT/HeT [H,S]: 2 psum m-tiles, 6 k-tiles each; ACT copy + per-partition
      bias -> bf16 SBUF tiles [128, 512]
  Ls   [s,c]: 4 s-tiles [128, 32] f32 (lhsT=HsT slice, rhs=W_s.T)
  LeWb [c,e]: [32, 512] bf16 (lhsT=W_e.T, rhs=HeT) + W_b bias
  per label c in 0..31:
    T_c[g,s]  = U[:,c,:].T-contracted with HsT   (2 m-tiles x 2 k-tiles)
    bc[p,e]   = indicator-matmul replicate of LeWb[c,:] across partitions
    biaff s-tile [128(s), 512(e)] = T_c.T @ HeT  (2 k-tiles)
    out tile  = DVE (psum + Ls[s,c]) + bc        (scalar_tensor_tensor)
    DMA -> out[c, s-tile, :]
"""

import sys

if "/opt/trn_rl_repo" not in sys.path:
    sys.path.insert(0, "/opt/trn_rl_repo")

import numpy as np
import ml_dtypes

BF16 = ml_dtypes.bfloat16

B, S, D = 8, 512, 768
H, C = 256, 32
P = 128
KD = D // P   # 6 k-tiles for the D contraction
KH = H // P   # 2 k-tiles for the H/G contraction
NS = S // P   # 4 s-tiles
N_CORES = 8

_CACHE: dict = {}


def _build_nc():
    from contextlib import ExitStack

    import concourse.tile as tile
    from concourse import bacc, mybir

    f32 = mybir.dt.float32
    bf16 = mybir.dt.bfloat16
    ID = mybir.ActivationFunctionType.Identity
    CP = mybir.ActivationFunctionType.Copy
    ADD = mybir.AluOpType.add

    nc = bacc.Bacc("TRN2", target_bir_lowering=False, debug=False,
                   num_devices=N_CORES)

    xT_d = nc.dram_tensor("xT", [D, S], bf16, kind="ExternalInput").ap()
    wswT_d = nc.dram_tensor("wswT", [D, H], bf16, kind="ExternalInput").ap()
    wewT_d = nc.dram_tensor("wewT", [D, H], bf16, kind="ExternalInput").ap()
    u_d = nc.dram_tensor("u", [H, C * H], bf16, kind="ExternalInput").ap()
    wsb_d = nc.dram_tensor("wsb", [KH, P, 1], f32, kind="ExternalInput").ap()
    web_d = nc.dram_tensor("web", [KH, P, 1], f32, kind="ExternalInput").ap()
    wstl_d = nc.dram_tensor("wstl", [H, C], bf16, kind="ExternalInput").ap()
    wetl_d = nc.dram_tensor("wetl", [H, C], bf16, kind="ExternalInput").ap()
    wb_d = nc.dram_tensor("wb", [C, 1], f32, kind="ExternalInput").ap()
    ind_d = nc.dram_tensor("ind", [C, C * P], bf16, kind="ExternalInput").ap()
    lewb_tmp_d = nc.dram_tensor("lewb_tmp", [1, C * S], f32).ap()
    out_d = nc.dram_tensor("out", [C, S, S], f32, kind="ExternalOutput").ap()

    with tile.TileContext(nc) as tc, ExitStack() as ctx:
        consts = ctx.enter_context(tc.tile_pool(name="consts", bufs=1))
        psum = ctx.enter_context(tc.tile_pool(name="psum", bufs=1, space="PSUM"))
        tcp = ctx.enter_context(tc.tile_pool(name="tcp", bufs=1))
        bcp = ctx.enter_context(tc.tile_pool(name="bcp", bufs=2))
        outp = ctx.enter_context(tc.tile_pool(name="outp", bufs=6))

        # ---- load constants / inputs into SBUF ----
        xT_r = xT_d.rearrange("(k p) s -> k p s", p=P)
        wswT_r = wswT_d.rearrange("(k p) h -> k p h", p=P)
        wewT_r = wewT_d.rearrange("(k p) h -> k p h", p=P)
        u_r = u_d.rearrange("(k p) f -> k p f", p=P)
        wstl_r = wstl_d.rearrange("(k p) c -> k p c", p=P)
        wetl_r = wetl_d.rearrange("(k p) c -> k p c", p=P)

        # Input DMA priority order, split across both queues. The chain to
        # the FIRST output tile is: xt+wsw -> HsT; U chunk 0 -> T_0;
        # wew -> HeT; wetl -> LeWb. Everything the first label needs goes
        # to the queue fronts; the U bulk streams behind it.
        xt, wsw, wew, usb, wstl, wetl = [], [], [], [], [], []
        for k in range(KD):
            q = nc.sync if k % 2 == 0 else nc.gpsimd
            t = consts.tile([P, S], bf16, tag=f"xt{k}", name=f"xt{k}")
            q.dma_start(t, xT_r[k])
            xt.append(t)
            t = consts.tile([P, H], bf16, tag=f"wsw{k}", name=f"wsw{k}")
            q.dma_start(t, wswT_r[k])
            wsw.append(t)
        UCH = 8
        UW = C * H // UCH
        for k in range(KH):
            t = consts.tile([P, C * H], bf16, tag=f"u{k}", name=f"u{k}")
            usb.append(t)
        ind_t = consts.tile([C, C * P], bf16, tag="ind", name="ind")
        nc.gpsimd.dma_start(ind_t, ind_d)
        for k in range(KH):
            nc.gpsimd.dma_start(usb[k][:, 0:UW], u_r[k][:, 0:UW])
        for k in range(KD):
            q = nc.sync if k % 2 == 0 else nc.gpsimd
            t = consts.tile([P, H], bf16, tag=f"wew{k}", name=f"wew{k}")
            q.dma_start(t, wewT_r[k])
            wew.append(t)
        wsb_t, web_t = [], []
        for m in range(KH):
            t = consts.tile([P, 1], f32, tag=f"wsb{m}", name=f"wsb{m}")
            nc.sync.dma_start(t, wsb_d[m])
            wsb_t.append(t)
            t = consts.tile([P, 1], f32, tag=f"web{m}", name=f"web{m}")
            nc.sync.dma_start(t, web_d[m])
            web_t.append(t)
        wb_t = consts.tile([C, 1], f32, tag="wb", name="wb")
        nc.sync.dma_start(wb_t, wb_d)
        for k in range(KH):
            t = consts.tile([P, C], bf16, tag=f"wstl{k}", name=f"wstl{k}")
            nc.sync.dma_start(t, wstl_r[k])
            wstl.append(t)
            t = consts.tile([P, C], bf16, tag=f"wetl{k}", name=f"wetl{k}")
            nc.sync.dma_start(t, wetl_r[k])
            wetl.append(t)
        # Remaining U bulk, split across both queues to finish it sooner.
        for j in range(1, UCH):
            for k in range(KH):
                q = nc.sync if (j * KH + k) % 2 == 0 else nc.gpsimd
                q.dma_start(usb[k][:, j * UW:(j + 1) * UW],
                            u_r[k][:, j * UW:(j + 1) * UW])
        # ---- stage A: projections HsT / HeT  [H, S] as 2 bf16 tiles ----
        hst, het = [], []
        for nm, wt, bias, dst in (("hs", wsw, wsb_t, hst),
                                  ("he", wew, web_t, het)):
            for m in range(KH):
                ps = psum.tile([P, S], f32, tag="psO", bufs=5,
                               name=f"ps_{nm}{m}")
                for k in range(KD):
                    nc.tensor.matmul(ps, lhsT=wt[k][:, m * P:(m + 1) * P],
                                     rhs=xt[k], start=(k == 0),
                                     stop=(k == KD - 1))
                ht = consts.tile([P, S], bf16, tag=f"{nm}t{m}",
                                 name=f"{nm}t{m}")
                nc.scalar.activation(ht, ps, ID, bias=bias[m])
                dst.append(ht)

        # ---- Ls s-tiles [128, 32] f32 ----
        ls_t = []
        for st in range(NS):
            ps = psum.tile([P, C], f32, tag="psT", bufs=3, name=f"ps_ls{st}")
            for k in range(KH):
                nc.tensor.matmul(ps, lhsT=hst[k][:, st * P:(st + 1) * P],
                                 rhs=wstl[k], start=(k == 0),
                                 stop=(k == KH - 1))
            lt = consts.tile([P, C], f32, tag=f"ls{st}", name=f"ls{st}")
            nc.vector.tensor_copy(lt, ps)
            ls_t.append(lt)

        # ---- LeWb [32, 512] bf16 (Le.T + W_b bias) ----
        ps = psum.tile([C, S], f32, tag="psT", bufs=3, name="ps_le")
        for k in range(KH):
            nc.tensor.matmul(ps, lhsT=wetl[k], rhs=het[k], start=(k == 0),
                             stop=(k == KH - 1))
        lewb = consts.tile([C, S], bf16, tag="lewb", name="lewb")
        nc.scalar.activation(lewb, ps, ID, bias=wb_t)
        # f32 LeWb, flattened onto partition 0 via a DRAM round-trip so
        # gpsimd.partition_broadcast (which requires base partition 0) can
        # replicate row c for the DVE-evicted s-tiles.
        lewb_f32 = consts.tile([C, S], f32, tag="lewbf", name="lewbf")
        nc.scalar.activation(lewb_f32, ps, ID, bias=wb_t)
        nc.sync.dma_start(
            lewb_tmp_d.rearrange("o (c s) -> (o c) s", s=S), lewb_f32)
        lewb_flat = consts.tile([1, C * S], f32, tag="lewbfl", name="lewbfl")
        nc.sync.dma_start(lewb_flat, lewb_tmp_d)

        # ---- stage B: software-pipelined per-label loop ----
        # Engine queues are in-order, so label c+1's T matmuls are emitted
        # BEFORE label c's biaff matmuls: while ACT evicts T_c from PSUM,
        # the PE streams T_{c+1} instead of stalling at the head of its
        # queue. The broadcast lin row (LeWb[c,:] replicated) is
        # materialized once per c into s-tile 0's PSUM bank (indicator
        # matmul); ACT snapshots it to SBUF (bc) for the DVE-evicted
        # s-tiles 1-3, then st 0's biaff accumulates on top in the same
        # bank and ACT evicts it with the Ls bias.
        po0s, psts = {}, {}

        def emit_front(c):
            po0 = psum.tile([P, S], f32, tag="psO", bufs=5,
                            name=f"ps_o{c}_0")
            nc.tensor.matmul(po0, lhsT=ind_t[:, c * P:(c + 1) * P],
                             rhs=lewb, start=True, stop=False)
            po0s[c] = po0
            pst = []
            for mg in range(KH):
                p_ = psum.tile([P, S], f32, tag="psT", bufs=3,
                               name=f"ps_t{c}_{mg}")
                off = c * H + mg * P
                for kh in range(KH):
                    nc.tensor.matmul(p_, lhsT=usb[kh][:, off:off + P],
                                     rhs=hst[kh], start=(kh == 0),
                                     stop=(kh == KH - 1))
                pst.append(p_)
            psts[c] = pst

        emit_front(0)
        for c in range(C):
            po0 = po0s.pop(c)
            bc = bcp.tile([P, S], f32, tag="bc", bufs=2, name=f"bc{c}")
            nc.gpsimd.partition_broadcast(bc, lewb_flat[0:1,
                                                        c * S:(c + 1) * S])
            tcb = tcp.tile([P, KH * S], bf16, tag="tc", bufs=2,
                           name=f"tc{c}")
            for mg in range(KH):
                nc.scalar.activation(tcb[:, mg * S:(mg + 1) * S],
                                     psts[c][mg], CP)
            del psts[c]
            if c + 1 < C:
                emit_front(c + 1)

            ot = outp.tile([P, NS * S], f32, tag="ot", bufs=4,
                           name=f"ot{c}")
            for st in range(1, NS):
                po = psum.tile([P, S], f32, tag="psO", bufs=5,
                               name=f"ps_o{c}_{st}")
                for kg in range(KH):
                    nc.tensor.matmul(po,
                                     lhsT=tcb[:, kg * S + st * P:
                                              kg * S + (st + 1) * P],
                                     rhs=het[kg], start=(kg == 0),
                                     stop=(kg == KH - 1))
                nc.vector.scalar_tensor_tensor(ot[:, st * S:(st + 1) * S],
                                               po, ls_t[st][:, c:c + 1], bc,
                                               op0=ADD, op1=ADD)
            # st 0 last: accumulate biaff onto the broadcast row already in
            # the bank (after ACT's snapshot read).
            for kg in range(KH):
                nc.tensor.matmul(po0, lhsT=tcb[:, kg * S:kg * S + P],
                                 rhs=het[kg], start=False,
                                 stop=(kg == KH - 1))
            nc.scalar.activation(ot[:, 0:S], po0, ID,
                                 bias=ls_t[0][:, c:c + 1])
            eng = nc.sync if c % 2 else nc.gpsimd
            dst = out_d[c].rearrange("(st p) e -> p st e", p=P)
            eng.dma_start(dst, ot.rearrange("p (st e) -> p st e", e=S))

    nc.compile()
    return nc


def _host_prep(seq_feats, U, W_w, W_b, Ws_w, Ws_b, We_w, We_b):
    """Build the per-core input maps (host-side layout prep, all small
    except seq_feats transpose)."""
    ind = np.zeros((C, C * P), dtype=BF16)
    for k in range(C):
        ind[k, k * P:(k + 1) * P] = 1.0

    common = {
        "wswT": np.ascontiguousarray(Ws_w.T).astype(BF16),
        "wewT": np.ascontiguousarray(We_w.T).astype(BF16),
        "u": np.ascontiguousarray(U.reshape(H, C * H)).astype(BF16),
        "wsb": np.ascontiguousarray(Ws_b.reshape(KH, P, 1)).astype(np.float32),
        "web": np.ascontiguousarray(We_b.reshape(KH, P, 1)).astype(np.float32),
        "wstl": np.ascontiguousarray(W_w[:, :H].T).astype(BF16),
        "wetl": np.ascontiguousarray(W_w[:, H:].T).astype(BF16),
        "wb": np.ascontiguousarray(W_b.reshape(C, 1)).astype(np.float32),
        "ind": ind,
    }
    in_maps = []
    for b in range(B):
        m = dict(common)
        m["xT"] = np.ascontiguousarray(seq_feats[b].T).astype(BF16)
        in_maps.append(m)
    return in_maps


def kernel(seq_feats, U, W_w, W_b, Ws_w, Ws_b, We_w, We_b):
    from concourse.bass_utils import run_bass_kernel_spmd

    seq_feats = np.asarray(seq_feats, dtype=np.float32)
    U = np.asarray(U, dtype=np.float32)
    W_w = np.asarray(W_w, dtype=np.float32)
    W_b = np.asarray(W_b, dtype=np.float32)
    Ws_w = np.asarray(Ws_w, dtype=np.float32)
    Ws_b = np.asarray(Ws_b, dtype=np.float32)
    We_w = np.asarray(We_w, dtype=np.float32)
    We_b = np.asarray(We_b, dtype=np.float32)

    if "nc" not in _CACHE:
        _CACHE["nc"] = _build_nc()
    nc = _CACHE["nc"]

    in_maps = _host_prep(seq_feats, U, W_w, W_b, Ws_w, Ws_b, We_w, We_b)
    res = run_bass_kernel_spmd(nc, in_maps, core_ids=list(range(N_CORES)))
    _CACHE["last_result"] = res

    out = np.stack([res.results[b]["out"] for b in range(B)])  # [B, C, S, S]
    return np.ascontiguousarray(out.transpose(0, 2, 3, 1)).astype(np.float32)


# revision 30
# speedup vs baseline: 1.0404x; 1.0404x over previous
"""Trainium2 Bass kernel for BiaffineSpanHead.

Math (per batch b):
  Hs = x @ Ws_w.T + Ws_b            [S, H]
  He = x @ We_w.T + We_b            [S, H]
  biaff[s,e,c] = sum_{h,g} Hs[s,h] U[h,c,g] He[e,g]
  out[s,e,c] = biaff + Ls[s,c] + Le[e,c] + W_b[c]
      Ls = Hs @ W_w[:, :H].T,  Le = He @ W_w[:, H:].T

Sharding: data-parallel over batch B=8 across 8 cores (one batch each).
Per-core device layout: out[c, s, e] (contiguous [32, 512, 512] f32);
host transposes back to [B, S, E, C].

Device dataflow per core (P=128 partitions):
  xT [D,S] -> 6 SBUF tiles [128, 512] bf16
  HsT/HeT [H,S]: 2 psum m-tiles, 6 k-tiles each; ACT copy + per-partition
      bias -> bf16 SBUF tiles [128, 512]
  Ls   [s,c]: 4 s-tiles [128, 32] f32 (lhsT=HsT slice, rhs=W_s.T)
  LeWb [c,e]: [32, 512] bf16 (lhsT=W_e.T, rhs=HeT) + W_b bias
  per label c in 0..31:
    T_c[g,s]  = U[:,c,:].T-contracted with HsT   (2 m-tiles x 2 k-tiles)
    bc[p,e]   = indicator-matmul replicate of LeWb[c,:] across partitions
    biaff s-tile [128(s), 512(e)] = T_c.T @ HeT  (2 k-tiles)
    out tile  = DVE (psum + Ls[s,c]) + bc        (scalar_tensor_tensor)
    DMA -> out[c, s-tile, :]
"""

import sys

if "/opt/trn_rl_repo" not in sys.path:
    sys.path.insert(0, "/opt/trn_rl_repo")

import numpy as np
import ml_dtypes

BF16 = ml_dtypes.bfloat16

B, S, D = 8, 512, 768
H, C = 256, 32
P = 128
KD = D // P   # 6 k-tiles for the D contraction
KH = H // P   # 2 k-tiles for the H/G contraction
NS = S // P   # 4 s-tiles
N_CORES = 8

_CACHE: dict = {}


def _build_nc():
    from contextlib import ExitStack

    import concourse.tile as tile
    from concourse import bacc, mybir

    f32 = mybir.dt.float32
    bf16 = mybir.dt.bfloat16
    ID = mybir.ActivationFunctionType.Identity
    CP = mybir.ActivationFunctionType.Copy
    ADD = mybir.AluOpType.add

    nc = bacc.Bacc("TRN2", target_bir_lowering=False, debug=False,
                   num_devices=N_CORES)

    xT_d = nc.dram_tensor("xT", [D, S], bf16, kind="ExternalInput").ap()
    wswT_d = nc.dram_tensor("wswT", [D, H], bf16, kind="ExternalInput").ap()
    wewT_d = nc.dram_tensor("wewT", [D, H], bf16, kind="ExternalInput").ap()
    u_d = nc.dram_tensor("u", [H, C * H], bf16, kind="ExternalInput").ap()
    wsb_d = nc.dram_tensor("wsb", [KH, P, 1], f32, kind="ExternalInput").ap()
    web_d = nc.dram_tensor("web", [KH, P, 1], f32, kind="ExternalInput").ap()
    wstl_d = nc.dram_tensor("wstl", [H, C], bf16, kind="ExternalInput").ap()
    wetl_d = nc.dram_tensor("wetl", [H, C], bf16, kind="ExternalInput").ap()
    wb_d = nc.dram_tensor("wb", [C, 1], f32, kind="ExternalInput").ap()
    ind_d = nc.dram_tensor("ind", [C, C * P], bf16, kind="ExternalInput").ap()
    out_d = nc.dram_tensor("out", [C, S, S], f32, kind="ExternalOutput").ap()

    with tile.TileContext(nc) as tc, ExitStack() as ctx:
        consts = ctx.enter_context(tc.tile_pool(name="consts", bufs=1))
        psum = ctx.enter_context(tc.tile_pool(name="psum", bufs=1, space="PSUM"))
        tcp = ctx.enter_context(tc.tile_pool(name="tcp", bufs=1))
        bcp = ctx.enter_context(tc.tile_pool(name="bcp", bufs=2))
        outp = ctx.enter_context(tc.tile_pool(name="outp", bufs=6))

        # ---- load constants / inputs into SBUF ----
        xT_r = xT_d.rearrange("(k p) s -> k p s", p=P)
        wswT_r = wswT_d.rearrange("(k p) h -> k p h", p=P)
        wewT_r = wewT_d.rearrange("(k p) h -> k p h", p=P)
        u_r = u_d.rearrange("(k p) f -> k p f", p=P)
        wstl_r = wstl_d.rearrange("(k p) c -> k p c", p=P)
        wetl_r = wetl_d.rearrange("(k p) c -> k p c", p=P)

        # Input DMA priority order, split across both queues. The chain to
        # the FIRST output tile is: xt+wsw -> HsT; U chunk 0 -> T_0;
        # wew -> HeT; wetl -> LeWb. Everything the first label needs goes
        # to the queue fronts; the U bulk streams behind it.
        xt, wsw, wew, usb, wstl, wetl = [], [], [], [], [], []
        for k in range(KD):
            q = nc.sync if k % 2 == 0 else nc.gpsimd
            t = consts.tile([P, S], bf16, tag=f"xt{k}", name=f"xt{k}")
            q.dma_start(t, xT_r[k])
            xt.append(t)
            t = consts.tile([P, H], bf16, tag=f"wsw{k}", name=f"wsw{k}")
            q.dma_start(t, wswT_r[k])
            wsw.append(t)
        UCH = 8
        UW = C * H // UCH
        for k in range(KH):
            t = consts.tile([P, C * H], bf16, tag=f"u{k}", name=f"u{k}")
            usb.append(t)
        ind_t = consts.tile([C, C * P], bf16, tag="ind", name="ind")
        nc.gpsimd.dma_start(ind_t, ind_d)
        for k in range(KH):
            nc.gpsimd.dma_start(usb[k][:, 0:UW], u_r[k][:, 0:UW])
        for k in range(KD):
            q = nc.sync if k % 2 == 0 else nc.gpsimd
            t = consts.tile([P, H], bf16, tag=f"wew{k}", name=f"wew{k}")
            q.dma_start(t, wewT_r[k])
            wew.append(t)
        wsb_t, web_t = [], []
        for m in range(KH):
            t = consts.tile([P, 1], f32, tag=f"wsb{m}", name=f"wsb{m}")
            nc.sync.dma_start(t, wsb_d[m])
            wsb_t.append(t)
            t = consts.tile([P, 1], f32, tag=f"web{m}", name=f"web{m}")
            nc.sync.dma_start(t, web_d[m])
            web_t.append(t)
        wb_t = consts.tile([C, 1], f32, tag="wb", name="wb")
        nc.sync.dma_start(wb_t, wb_d)
        for k in range(KH):
            t = consts.tile([P, C], bf16, tag=f"wstl{k}", name=f"wstl{k}")
            nc.sync.dma_start(t, wstl_r[k])
            wstl.append(t)
            t = consts.tile([P, C], bf16, tag=f"wetl{k}", name=f"wetl{k}")
            nc.sync.dma_start(t, wetl_r[k])
            wetl.append(t)
        # Remaining U bulk, split across both queues to finish it sooner.
        for j in range(1, UCH):
            for k in range(KH):
                q = nc.sync if (j * KH + k) % 2 == 0 else nc.gpsimd
                q.dma_start(usb[k][:, j * UW:(j + 1) * UW],
                            u_r[k][:, j * UW:(j + 1) * UW])
        # ---- stage A: projections HsT / HeT  [H, S] as 2 bf16 tiles ----
        hst, het = [], []
        for nm, wt, bias, dst in (("hs", wsw, wsb_t, hst),
                                  ("he", wew, web_t, het)):
            for m in range(KH):
                ps = psum.tile([P, S], f32, tag="psO", bufs=5,
                               name=f"ps_{nm}{m}")
                for k in range(KD):
                    nc.tensor.matmul(ps, lhsT=wt[k][:, m * P:(m + 1) * P],
                                     rhs=xt[k], start=(k == 0),
                                     stop=(k == KD - 1))
                ht = consts.tile([P, S], bf16, tag=f"{nm}t{m}",
                                 name=f"{nm}t{m}")
                nc.scalar.activation(ht, ps, ID, bias=bias[m])
                dst.append(ht)

        # ---- Ls s-tiles [128, 32] f32 ----
        ls_t = []
        for st in range(NS):
            ps = psum.tile([P, C], f32, tag="psT", bufs=3, name=f"ps_ls{st}")
            for k in range(KH):
                nc.tensor.matmul(ps, lhsT=hst[k][:, st * P:(st + 1) * P],
                                 rhs=wstl[k], start=(k == 0),
                                 stop=(k == KH - 1))
            lt = consts.tile([P, C], f32, tag=f"ls{st}", name=f"ls{st}")
            nc.vector.tensor_copy(lt, ps)
            ls_t.append(lt)

        # ---- LeWb [32, 512] bf16 (Le.T + W_b bias) ----
        ps = psum.tile([C, S], f32, tag="psT", bufs=3, name="ps_le")
        for k in range(KH):
            nc.tensor.matmul(ps, lhsT=wetl[k], rhs=het[k], start=(k == 0),
                             stop=(k == KH - 1))
        lewb = consts.tile([C, S], bf16, tag="lewb", name="lewb")
        nc.scalar.activation(lewb, ps, ID, bias=wb_t)


        # ---- stage B: software-pipelined per-label loop ----
        # Engine queues are in-order, so label c+1's T matmuls are emitted
        # BEFORE label c's biaff matmuls: while ACT evicts T_c from PSUM,
        # the PE streams T_{c+1} instead of stalling at the head of its
        # queue. The broadcast lin row (LeWb[c,:] replicated) is
        # materialized once per c into s-tile 0's PSUM bank (indicator
        # matmul); ACT snapshots it to SBUF (bc) for the DVE-evicted
        # s-tiles 1-3, then st 0's biaff accumulates on top in the same
        # bank and ACT evicts it with the Ls bias.
        po0s, psts = {}, {}

        def emit_front(c):
            po0 = psum.tile([P, S], f32, tag="psO", bufs=5,
                            name=f"ps_o{c}_0")
            nc.tensor.matmul(po0, lhsT=ind_t[:, c * P:(c + 1) * P],
                             rhs=lewb, start=True, stop=False)
            po0s[c] = po0
            pst = []
            for mg in range(KH):
                p_ = psum.tile([P, S], f32, tag="psT", bufs=3,
                               name=f"ps_t{c}_{mg}")
                off = c * H + mg * P
                for kh in range(KH):
                    nc.tensor.matmul(p_, lhsT=usb[kh][:, off:off + P],
                                     rhs=hst[kh], start=(kh == 0),
                                     stop=(kh == KH - 1))
                pst.append(p_)
            psts[c] = pst

        emit_front(0)
        for c in range(C):
            po0 = po0s.pop(c)
            bc = bcp.tile([P, S], f32, tag="bc", bufs=2, name=f"bc{c}")
            nc.scalar.activation(bc, po0, CP)
            tcb = tcp.tile([P, KH * S], bf16, tag="tc", bufs=2,
                           name=f"tc{c}")
            for mg in range(KH):
                nc.scalar.activation(tcb[:, mg * S:(mg + 1) * S],
                                     psts[c][mg], CP)
            del psts[c]
            if c + 1 < C:
                emit_front(c + 1)

            ot = outp.tile([P, NS * S], f32, tag="ot", bufs=4,
                           name=f"ot{c}")
            for st in range(1, NS):
                po = psum.tile([P, S], f32, tag="psO", bufs=5,
                               name=f"ps_o{c}_{st}")
                for kg in range(KH):
                    nc.tensor.matmul(po,
                                     lhsT=tcb[:, kg * S + st * P:
                                              kg * S + (st + 1) * P],
                                     rhs=het[kg], start=(kg == 0),
                                     stop=(kg == KH - 1))
                nc.vector.scalar_tensor_tensor(ot[:, st * S:(st + 1) * S],
                                               po, ls_t[st][:, c:c + 1], bc,
                                               op0=ADD, op1=ADD)
            # st 0 last: accumulate biaff onto the broadcast row already in
            # the bank (after ACT's snapshot read).
            for kg in range(KH):
                nc.tensor.matmul(po0, lhsT=tcb[:, kg * S:kg * S + P],
                                 rhs=het[kg], start=False,
                                 stop=(kg == KH - 1))
            nc.scalar.activation(ot[:, 0:S], po0, ID,
                                 bias=ls_t[0][:, c:c + 1])
            eng = nc.sync if c % 2 else nc.gpsimd
            dst = out_d[c].rearrange("(st p) e -> p st e", p=P)
            eng.dma_start(dst, ot.rearrange("p (st e) -> p st e", e=S))

    nc.compile()
    return nc


def _host_prep(seq_feats, U, W_w, W_b, Ws_w, Ws_b, We_w, We_b):
    """Build the per-core input maps (host-side layout prep, all small
    except seq_feats transpose)."""
    ind = np.zeros((C, C * P), dtype=BF16)
    for k in range(C):
        ind[k, k * P:(k + 1) * P] = 1.0

    common = {
        "wswT": np.ascontiguousarray(Ws_w.T).astype(BF16),
        "wewT": np.ascontiguousarray(We_w.T).astype(BF16),
        "u": np.ascontiguousarray(U.reshape(H, C * H)).astype(BF16),
        "wsb": np.ascontiguousarray(Ws_b.reshape(KH, P, 1)).astype(np.float32),
        "web": np.ascontiguousarray(We_b.reshape(KH, P, 1)).astype(np.float32),
        "wstl": np.ascontiguousarray(W_w[:, :H].T).astype(BF16),
        "wetl": np.ascontiguousarray(W_w[:, H:].T).astype(BF16),
        "wb": np.ascontiguousarray(W_b.reshape(C, 1)).astype(np.float32),
        "ind": ind,
    }
    in_maps = []
    for b in range(B):
        m = dict(common)
        m["xT"] = np.ascontiguousarray(seq_feats[b].T).astype(BF16)
        in_maps.append(m)
    return in_maps


def kernel(seq_feats, U, W_w, W_b, Ws_w, Ws_b, We_w, We_b):
    from concourse.bass_utils import run_bass_kernel_spmd

    seq_feats = np.asarray(seq_feats, dtype=np.float32)
    U = np.asarray(U, dtype=np.float32)
    W_w = np.asarray(W_w, dtype=np.float32)
    W_b = np.asarray(W_b, dtype=np.float32)
    Ws_w = np.asarray(Ws_w, dtype=np.float32)
    Ws_b = np.asarray(Ws_b, dtype=np.float32)
    We_w = np.asarray(We_w, dtype=np.float32)
    We_b = np.asarray(We_b, dtype=np.float32)

    if "nc" not in _CACHE:
        _CACHE["nc"] = _build_nc()
    nc = _CACHE["nc"]

    in_maps = _host_prep(seq_feats, U, W_w, W_b, Ws_w, Ws_b, We_w, We_b)
    res = run_bass_kernel_spmd(nc, in_maps, core_ids=list(range(N_CORES)))
    _CACHE["last_result"] = res

    out = np.stack([res.results[b]["out"] for b in range(B)])  # [B, C, S, S]
    return np.ascontiguousarray(out.transpose(0, 2, 3, 1)).astype(np.float32)


# revision 35
# speedup vs baseline: 1.0452x; 1.0046x over previous
"""Trainium2 Bass kernel for BiaffineSpanHead.

Math (per batch b):
  Hs = x @ Ws_w.T + Ws_b            [S, H]
  He = x @ We_w.T + We_b            [S, H]
  biaff[s,e,c] = sum_{h,g} Hs[s,h] U[h,c,g] He[e,g]
  out[s,e,c] = biaff + Ls[s,c] + Le[e,c] + W_b[c]
      Ls = Hs @ W_w[:, :H].T,  Le = He @ W_w[:, H:].T

Sharding: data-parallel over batch B=8 across 8 cores (one batch each).
Per-core device layout: out[c, s, e] (contiguous [32, 512, 512] f32);
host transposes back to [B, S, E, C].

Device dataflow per core (P=128 partitions):
  xT [D,S] -> 6 SBUF tiles [128, 512] bf16
  HsT/HeT [H,S]: 2 psum m-tiles, 6 k-tiles each; ACT copy + per-partition
      bias -> bf16 SBUF tiles [128, 512]
  Ls   [s,c]: 4 s-tiles [128, 32] f32 (lhsT=HsT slice, rhs=W_s.T)
  LeWb [c,e]: [32, 512] bf16 (lhsT=W_e.T, rhs=HeT) + W_b bias
  per label c in 0..31:
    T_c[g,s]  = U[:,c,:].T-contracted with HsT   (2 m-tiles x 2 k-tiles)
    bc[p,e]   = indicator-matmul replicate of LeWb[c,:] across partitions
    biaff s-tile [128(s), 512(e)] = T_c.T @ HeT  (2 k-tiles)
    out tile  = DVE (psum + Ls[s,c]) + bc        (scalar_tensor_tensor)
    DMA -> out[c, s-tile, :]
"""

import sys

if "/opt/trn_rl_repo" not in sys.path:
    sys.path.insert(0, "/opt/trn_rl_repo")

import numpy as np
import ml_dtypes

BF16 = ml_dtypes.bfloat16

B, S, D = 8, 512, 768
H, C = 256, 32
P = 128
KD = D // P   # 6 k-tiles for the D contraction
KH = H // P   # 2 k-tiles for the H/G contraction
NS = S // P   # 4 s-tiles
N_CORES = 8

_CACHE: dict = {}


def _build_nc():
    from contextlib import ExitStack

    import concourse.tile as tile
    from concourse import bacc, mybir

    f32 = mybir.dt.float32
    bf16 = mybir.dt.bfloat16
    ID = mybir.ActivationFunctionType.Identity
    CP = mybir.ActivationFunctionType.Copy
    ADD = mybir.AluOpType.add

    nc = bacc.Bacc("TRN2", target_bir_lowering=False, debug=False,
                   num_devices=N_CORES)

    # xT | Ws_w.T | We_w.T packed host-side into one [D, S+2H] array so the
    # projection-critical inputs arrive in 6 big DMAs.
    pin_d = nc.dram_tensor("pin", [D, S + 2 * H], bf16,
                           kind="ExternalInput").ap()
    u_d = nc.dram_tensor("u", [H, C * H], bf16, kind="ExternalInput").ap()
    wsb_d = nc.dram_tensor("wsb", [KH, P, 1], f32, kind="ExternalInput").ap()
    web_d = nc.dram_tensor("web", [KH, P, 1], f32, kind="ExternalInput").ap()
    wstl_d = nc.dram_tensor("wstl", [H, C], bf16, kind="ExternalInput").ap()
    wetl_d = nc.dram_tensor("wetl", [H, C], bf16, kind="ExternalInput").ap()
    wb_d = nc.dram_tensor("wb", [C, 1], f32, kind="ExternalInput").ap()
    ind_d = nc.dram_tensor("ind", [C, C * P], bf16, kind="ExternalInput").ap()
    out_d = nc.dram_tensor("out", [C, S, S], f32, kind="ExternalOutput").ap()

    with tile.TileContext(nc) as tc, ExitStack() as ctx:
        consts = ctx.enter_context(tc.tile_pool(name="consts", bufs=1))
        psum = ctx.enter_context(tc.tile_pool(name="psum", bufs=1, space="PSUM"))
        tcp = ctx.enter_context(tc.tile_pool(name="tcp", bufs=1))
        bcp = ctx.enter_context(tc.tile_pool(name="bcp", bufs=2))
        outp = ctx.enter_context(tc.tile_pool(name="outp", bufs=6))

        # ---- load constants / inputs into SBUF ----
        pin_r = pin_d.rearrange("(k p) f -> k p f", p=P)
        u_r = u_d.rearrange("(k p) f -> k p f", p=P)
        wstl_r = wstl_d.rearrange("(k p) c -> k p c", p=P)
        wetl_r = wetl_d.rearrange("(k p) c -> k p c", p=P)

        # Input DMA priority order, split across both queues. The chain to
        # the FIRST output tile is: xt+wsw -> HsT; wew -> HeT -> LeWb;
        # U chunk 0 -> T_0. All projection inputs ride in 6 packed DMAs
        # ([128, 1024]: xT | Ws | We per k-tile); the U bulk streams
        # behind them.
        xt, wsw, wew, usb, wstl, wetl = [], [], [], [], [], []
        pin_t = []
        for k in range(KD):
            q = nc.sync if k % 2 == 0 else nc.gpsimd
            t = consts.tile([P, S + 2 * H], bf16, tag=f"pin{k}",
                            name=f"pin{k}")
            q.dma_start(t, pin_r[k])
            pin_t.append(t)
            xt.append(t[:, 0:S])
            wsw.append(t[:, S:S + H])
            wew.append(t[:, S + H:S + 2 * H])
        UCH = 8
        UW = C * H // UCH
        for k in range(KH):
            t = consts.tile([P, C * H], bf16, tag=f"u{k}", name=f"u{k}")
            usb.append(t)
        ind_t = consts.tile([C, C * P], bf16, tag="ind", name="ind")
        nc.gpsimd.dma_start(ind_t, ind_d)
        for k in range(KH):
            nc.gpsimd.dma_start(usb[k][:, 0:UW], u_r[k][:, 0:UW])
        wsb_t, web_t = [], []
        for m in range(KH):
            t = consts.tile([P, 1], f32, tag=f"wsb{m}", name=f"wsb{m}")
            nc.sync.dma_start(t, wsb_d[m])
            wsb_t.append(t)
            t = consts.tile([P, 1], f32, tag=f"web{m}", name=f"web{m}")
            nc.sync.dma_start(t, web_d[m])
            web_t.append(t)
        wb_t = consts.tile([C, 1], f32, tag="wb", name="wb")
        nc.sync.dma_start(wb_t, wb_d)
        for k in range(KH):
            t = consts.tile([P, C], bf16, tag=f"wstl{k}", name=f"wstl{k}")
            nc.sync.dma_start(t, wstl_r[k])
            wstl.append(t)
            t = consts.tile([P, C], bf16, tag=f"wetl{k}", name=f"wetl{k}")
            nc.sync.dma_start(t, wetl_r[k])
            wetl.append(t)
        # Remaining U bulk, split across both queues to finish it sooner.
        for j in range(1, UCH):
            for k in range(KH):
                q = nc.sync if (j * KH + k) % 2 == 0 else nc.gpsimd
                q.dma_start(usb[k][:, j * UW:(j + 1) * UW],
                            u_r[k][:, j * UW:(j + 1) * UW])
        # ---- stage A: projections HsT / HeT  [H, S] as 2 bf16 tiles ----
        hst, het = [], []
        for nm, wt, bias, dst in (("hs", wsw, wsb_t, hst),
                                  ("he", wew, web_t, het)):
            for m in range(KH):
                ps = psum.tile([P, S], f32, tag="psO", bufs=5,
                               name=f"ps_{nm}{m}")
                for k in range(KD):
                    nc.tensor.matmul(ps, lhsT=wt[k][:, m * P:(m + 1) * P],
                                     rhs=xt[k], start=(k == 0),
                                     stop=(k == KD - 1))
                ht = consts.tile([P, S], bf16, tag=f"{nm}t{m}",
                                 name=f"{nm}t{m}")
                nc.scalar.activation(ht, ps, ID, bias=bias[m])
                dst.append(ht)

        # ---- Ls s-tiles [128, 32] f32 ----
        ls_t = []
        for st in range(NS):
            ps = psum.tile([P, C], f32, tag="psT", bufs=3, name=f"ps_ls{st}")
            for k in range(KH):
                nc.tensor.matmul(ps, lhsT=hst[k][:, st * P:(st + 1) * P],
                                 rhs=wstl[k], start=(k == 0),
                                 stop=(k == KH - 1))
            lt = consts.tile([P, C], f32, tag=f"ls{st}", name=f"ls{st}")
            nc.vector.tensor_copy(lt, ps)
            ls_t.append(lt)

        # ---- LeWb [32, 512] bf16 (Le.T + W_b bias) ----
        ps = psum.tile([C, S], f32, tag="psT", bufs=3, name="ps_le")
        for k in range(KH):
            nc.tensor.matmul(ps, lhsT=wetl[k], rhs=het[k], start=(k == 0),
                             stop=(k == KH - 1))
        lewb = consts.tile([C, S], bf16, tag="lewb", name="lewb")
        nc.scalar.activation(lewb, ps, ID, bias=wb_t)


        # ---- stage B: software-pipelined per-label loop ----
        # Engine queues are in-order, so label c+1's T matmuls are emitted
        # BEFORE label c's biaff matmuls: while ACT evicts T_c from PSUM,
        # the PE streams T_{c+1} instead of stalling at the head of its
        # queue. The broadcast lin row (LeWb[c,:] replicated) is
        # materialized once per c into s-tile 0's PSUM bank (indicator
        # matmul); ACT snapshots it to SBUF (bc) for the DVE-evicted
        # s-tiles 1-3, then st 0's biaff accumulates on top in the same
        # bank and ACT evicts it with the Ls bias.
        po0s, psts = {}, {}

        def emit_front(c):
            po0 = psum.tile([P, S], f32, tag="psO", bufs=5,
                            name=f"ps_o{c}_0")
            nc.tensor.matmul(po0, lhsT=ind_t[:, c * P:(c + 1) * P],
                             rhs=lewb, start=True, stop=False)
            po0s[c] = po0
            pst = []
            for mg in range(KH):
                p_ = psum.tile([P, S], f32, tag="psT", bufs=3,
                               name=f"ps_t{c}_{mg}")
                off = c * H + mg * P
                for kh in range(KH):
                    nc.tensor.matmul(p_, lhsT=usb[kh][:, off:off + P],
                                     rhs=hst[kh], start=(kh == 0),
                                     stop=(kh == KH - 1))
                pst.append(p_)
            psts[c] = pst

        emit_front(0)
        for c in range(C):
            po0 = po0s.pop(c)
            bc = bcp.tile([P, S], f32, tag="bc", bufs=2, name=f"bc{c}")
            nc.scalar.activation(bc, po0, CP)
            tcb = tcp.tile([P, KH * S], bf16, tag="tc", bufs=2,
                           name=f"tc{c}")
            for mg in range(KH):
                nc.scalar.activation(tcb[:, mg * S:(mg + 1) * S],
                                     psts[c][mg], CP)
            del psts[c]
            if c + 1 < C:
                emit_front(c + 1)

            ot = outp.tile([P, NS * S], f32, tag="ot", bufs=4,
                           name=f"ot{c}")
            for st in range(1, NS):
                po = psum.tile([P, S], f32, tag="psO", bufs=5,
                               name=f"ps_o{c}_{st}")
                for kg in range(KH):
                    nc.tensor.matmul(po,
                                     lhsT=tcb[:, kg * S + st * P:
                                              kg * S + (st + 1) * P],
                                     rhs=het[kg], start=(kg == 0),
                                     stop=(kg == KH - 1))
                nc.vector.scalar_tensor_tensor(ot[:, st * S:(st + 1) * S],
                                               po, ls_t[st][:, c:c + 1], bc,
                                               op0=ADD, op1=ADD)
            # st 0 last: accumulate biaff onto the broadcast row already in
            # the bank (after ACT's snapshot read).
            for kg in range(KH):
                nc.tensor.matmul(po0, lhsT=tcb[:, kg * S:kg * S + P],
                                 rhs=het[kg], start=False,
                                 stop=(kg == KH - 1))
            nc.scalar.activation(ot[:, 0:S], po0, ID,
                                 bias=ls_t[0][:, c:c + 1])
            eng = nc.sync if c % 2 else nc.gpsimd
            dst = out_d[c].rearrange("(st p) e -> p st e", p=P)
            eng.dma_start(dst, ot.rearrange("p (st e) -> p st e", e=S))

    nc.compile()
    return nc


def _host_prep(seq_feats, U, W_w, W_b, Ws_w, Ws_b, We_w, We_b):
    """Build the per-core input maps (host-side layout prep, all small
    except seq_feats transpose)."""
    ind = np.zeros((C, C * P), dtype=BF16)
    for k in range(C):
        ind[k, k * P:(k + 1) * P] = 1.0

    pin_w = np.concatenate([Ws_w.T, We_w.T], axis=1).astype(BF16)  # [D, 2H]
    common = {
        "u": np.ascontiguousarray(U.reshape(H, C * H)).astype(BF16),
        "wsb": np.ascontiguousarray(Ws_b.reshape(KH, P, 1)).astype(np.float32),
        "web": np.ascontiguousarray(We_b.reshape(KH, P, 1)).astype(np.float32),
        "wstl": np.ascontiguousarray(W_w[:, :H].T).astype(BF16),
        "wetl": np.ascontiguousarray(W_w[:, H:].T).astype(BF16),
        "wb": np.ascontiguousarray(W_b.reshape(C, 1)).astype(np.float32),
        "ind": ind,
    }
    in_maps = []
    for b in range(B):
        m = dict(common)
        pin = np.empty((D, S + 2 * H), dtype=BF16)
        pin[:, :S] = seq_feats[b].T
        pin[:, S:] = pin_w
        m["pin"] = pin
        in_maps.append(m)
    return in_maps


def kernel(seq_feats, U, W_w, W_b, Ws_w, Ws_b, We_w, We_b):
    from concourse.bass_utils import run_bass_kernel_spmd

    seq_feats = np.asarray(seq_feats, dtype=np.float32)
    U = np.asarray(U, dtype=np.float32)
    W_w = np.asarray(W_w, dtype=np.float32)
    W_b = np.asarray(W_b, dtype=np.float32)
    Ws_w = np.asarray(Ws_w, dtype=np.float32)
    Ws_b = np.asarray(Ws_b, dtype=np.float32)
    We_w = np.asarray(We_w, dtype=np.float32)
    We_b = np.asarray(We_b, dtype=np.float32)

    if "nc" not in _CACHE:
        _CACHE["nc"] = _build_nc()
    nc = _CACHE["nc"]

    in_maps = _host_prep(seq_feats, U, W_w, W_b, Ws_w, Ws_b, We_w, We_b)
    res = run_bass_kernel_spmd(nc, in_maps, core_ids=list(range(N_CORES)))
    _CACHE["last_result"] = res

    out = np.stack([res.results[b]["out"] for b in range(B)])  # [B, C, S, S]
    return np.ascontiguousarray(out.transpose(0, 2, 3, 1)).astype(np.float32)


# revision 37
# speedup vs baseline: 1.0532x; 1.0077x over previous
"""Trainium2 Bass kernel for BiaffineSpanHead.

Math (per batch b):
  Hs = x @ Ws_w.T + Ws_b            [S, H]
  He = x @ We_w.T + We_b            [S, H]
  biaff[s,e,c] = sum_{h,g} Hs[s,h] U[h,c,g] He[e,g]
  out[s,e,c] = biaff + Ls[s,c] + Le[e,c] + W_b[c]
      Ls = Hs @ W_w[:, :H].T,  Le = He @ W_w[:, H:].T

Sharding: data-parallel over batch B=8 across 8 cores (one batch each).
Per-core device layout: out[c, s, e] (contiguous [32, 512, 512] f32);
host transposes back to [B, S, E, C].

Device dataflow per core (P=128 partitions):
  xT [D,S] -> 6 SBUF tiles [128, 512] bf16
  HsT/HeT [H,S]: 2 psum m-tiles, 6 k-tiles each; ACT copy + per-partition
      bias -> bf16 SBUF tiles [128, 512]
  Ls   [s,c]: 4 s-tiles [128, 32] f32 (lhsT=HsT slice, rhs=W_s.T)
  LeWb [c,e]: [32, 512] bf16 (lhsT=W_e.T, rhs=HeT) + W_b bias
  per label c in 0..31:
    T_c[g,s]  = U[:,c,:].T-contracted with HsT   (2 m-tiles x 2 k-tiles)
    bc[p,e]   = indicator-matmul replicate of LeWb[c,:] across partitions
    biaff s-tile [128(s), 512(e)] = T_c.T @ HeT  (2 k-tiles)
    out tile  = DVE (psum + Ls[s,c]) + bc        (scalar_tensor_tensor)
    DMA -> out[c, s-tile, :]
"""

import sys

if "/opt/trn_rl_repo" not in sys.path:
    sys.path.insert(0, "/opt/trn_rl_repo")

import numpy as np
import ml_dtypes

BF16 = ml_dtypes.bfloat16

B, S, D = 8, 512, 768
H, C = 256, 32
P = 128
KD = D // P   # 6 k-tiles for the D contraction
KH = H // P   # 2 k-tiles for the H/G contraction
NS = S // P   # 4 s-tiles
N_CORES = 8

_CACHE: dict = {}


def _build_nc():
    from contextlib import ExitStack

    import concourse.tile as tile
    from concourse import bacc, mybir

    f32 = mybir.dt.float32
    bf16 = mybir.dt.bfloat16
    ID = mybir.ActivationFunctionType.Identity
    CP = mybir.ActivationFunctionType.Copy
    ADD = mybir.AluOpType.add

    nc = bacc.Bacc("TRN2", target_bir_lowering=False, debug=False,
                   num_devices=N_CORES)

    # xT | Ws_w.T | We_w.T packed host-side into one [D, S+2H] array so the
    # projection-critical inputs arrive in 6 big DMAs.
    pin_d = nc.dram_tensor("pin", [D, S + 2 * H], bf16,
                           kind="ExternalInput").ap()
    u_d = nc.dram_tensor("u", [H, C * H], bf16, kind="ExternalInput").ap()
    wsb_d = nc.dram_tensor("wsb", [KH, P, 1], f32, kind="ExternalInput").ap()
    web_d = nc.dram_tensor("web", [KH, P, 1], f32, kind="ExternalInput").ap()
    wstl_d = nc.dram_tensor("wstl", [H, C], bf16, kind="ExternalInput").ap()
    wetl_d = nc.dram_tensor("wetl", [H, C], bf16, kind="ExternalInput").ap()
    wb_d = nc.dram_tensor("wb", [C, 1], f32, kind="ExternalInput").ap()
    ind_d = nc.dram_tensor("ind", [C, C * P], bf16, kind="ExternalInput").ap()
    out_d = nc.dram_tensor("out", [C, S, S], f32, kind="ExternalOutput").ap()

    with tile.TileContext(nc) as tc, ExitStack() as ctx:
        consts = ctx.enter_context(tc.tile_pool(name="consts", bufs=1))
        psum = ctx.enter_context(tc.tile_pool(name="psum", bufs=1, space="PSUM"))
        tcp = ctx.enter_context(tc.tile_pool(name="tcp", bufs=1))
        bcp = ctx.enter_context(tc.tile_pool(name="bcp", bufs=2))
        outp = ctx.enter_context(tc.tile_pool(name="outp", bufs=6))

        # ---- load constants / inputs into SBUF ----
        pin_r = pin_d.rearrange("(k p) f -> k p f", p=P)
        u_r = u_d.rearrange("(k p) f -> k p f", p=P)
        wstl_r = wstl_d.rearrange("(k p) c -> k p c", p=P)
        wetl_r = wetl_d.rearrange("(k p) c -> k p c", p=P)

        # Input DMA priority order, split across both queues. The chain to
        # the FIRST output tile is: xt+wsw -> HsT; wew -> HeT -> LeWb;
        # U chunk 0 -> T_0. All projection inputs ride in 6 packed DMAs
        # ([128, 1024]: xT | Ws | We per k-tile); the U bulk streams
        # behind them.
        xt, wsw, wew, usb, wstl, wetl = [], [], [], [], [], []
        pin_t = []
        for k in range(KD):
            q = nc.sync if k % 2 == 0 else nc.gpsimd
            t = consts.tile([P, S + 2 * H], bf16, tag=f"pin{k}",
                            name=f"pin{k}")
            q.dma_start(t, pin_r[k])
            pin_t.append(t)
            xt.append(t[:, 0:S])
            wsw.append(t[:, S:S + H])
            wew.append(t[:, S + H:S + 2 * H])
        UCH = 8
        UW = C * H // UCH
        for k in range(KH):
            t = consts.tile([P, C * H], bf16, tag=f"u{k}", name=f"u{k}")
            usb.append(t)
        ind_t = consts.tile([C, C * P], bf16, tag="ind", name="ind")
        nc.gpsimd.dma_start(ind_t, ind_d)
        for k in range(KH):
            nc.gpsimd.dma_start(usb[k][:, 0:UW], u_r[k][:, 0:UW])
        wsb_t, web_t = [], []
        for m in range(KH):
            t = consts.tile([P, 1], f32, tag=f"wsb{m}", name=f"wsb{m}")
            nc.sync.dma_start(t, wsb_d[m])
            wsb_t.append(t)
            t = consts.tile([P, 1], f32, tag=f"web{m}", name=f"web{m}")
            nc.sync.dma_start(t, web_d[m])
            web_t.append(t)
        wb_t = consts.tile([C, 1], f32, tag="wb", name="wb")
        nc.sync.dma_start(wb_t, wb_d)
        for k in range(KH):
            t = consts.tile([P, C], bf16, tag=f"wstl{k}", name=f"wstl{k}")
            nc.sync.dma_start(t, wstl_r[k])
            wstl.append(t)
            t = consts.tile([P, C], bf16, tag=f"wetl{k}", name=f"wetl{k}")
            nc.sync.dma_start(t, wetl_r[k])
            wetl.append(t)
        # Remaining U bulk, split across both queues to finish it sooner.
        for j in range(1, UCH):
            for k in range(KH):
                q = nc.sync if (j * KH + k) % 2 == 0 else nc.gpsimd
                q.dma_start(usb[k][:, j * UW:(j + 1) * UW],
                            u_r[k][:, j * UW:(j + 1) * UW])
        # PE warmup: dummy matmuls with no input dependencies fill the
        # preamble dead time (PE would idle ~10 us waiting for the first
        # DMA) so the HAM clock-gate is at 8/8 when real matmuls start.
        warm_t = consts.tile([P, S], bf16, tag="warm", name="warm")
        nc.vector.memset(warm_t, 0.0)
        wps = psum.tile([P, S], f32, tag="psT", bufs=3, name="ps_warm")
        for _ in range(18):
            nc.tensor.matmul(wps, lhsT=warm_t[:, 0:P], rhs=warm_t,
                             start=True, stop=True)

        # ---- stage A: projections HsT / HeT  [H, S] as 2 bf16 tiles ----
        hst, het = [], []
        for nm, wt, bias, dst in (("hs", wsw, wsb_t, hst),
                                  ("he", wew, web_t, het)):
            for m in range(KH):
                ps = psum.tile([P, S], f32, tag="psO", bufs=5,
                               name=f"ps_{nm}{m}")
                for k in range(KD):
                    nc.tensor.matmul(ps, lhsT=wt[k][:, m * P:(m + 1) * P],
                                     rhs=xt[k], start=(k == 0),
                                     stop=(k == KD - 1))
                ht = consts.tile([P, S], bf16, tag=f"{nm}t{m}",
                                 name=f"{nm}t{m}")
                nc.scalar.activation(ht, ps, ID, bias=bias[m])
                dst.append(ht)

        # ---- Ls s-tiles [128, 32] f32 ----
        ls_t = []
        for st in range(NS):
            ps = psum.tile([P, C], f32, tag="psT", bufs=3, name=f"ps_ls{st}")
            for k in range(KH):
                nc.tensor.matmul(ps, lhsT=hst[k][:, st * P:(st + 1) * P],
                                 rhs=wstl[k], start=(k == 0),
                                 stop=(k == KH - 1))
            lt = consts.tile([P, C], f32, tag=f"ls{st}", name=f"ls{st}")
            nc.vector.tensor_copy(lt, ps)
            ls_t.append(lt)

        # ---- LeWb [32, 512] bf16 (Le.T + W_b bias) ----
        ps = psum.tile([C, S], f32, tag="psT", bufs=3, name="ps_le")
        for k in range(KH):
            nc.tensor.matmul(ps, lhsT=wetl[k], rhs=het[k], start=(k == 0),
                             stop=(k == KH - 1))
        lewb = consts.tile([C, S], bf16, tag="lewb", name="lewb")
        nc.scalar.activation(lewb, ps, ID, bias=wb_t)


        # ---- stage B: software-pipelined per-label loop ----
        # Engine queues are in-order, so label c+1's T matmuls are emitted
        # BEFORE label c's biaff matmuls: while ACT evicts T_c from PSUM,
        # the PE streams T_{c+1} instead of stalling at the head of its
        # queue. The broadcast lin row (LeWb[c,:] replicated) is
        # materialized once per c into s-tile 0's PSUM bank (indicator
        # matmul); ACT snapshots it to SBUF (bc) for the DVE-evicted
        # s-tiles 1-3, then st 0's biaff accumulates on top in the same
        # bank and ACT evicts it with the Ls bias.
        po0s, psts = {}, {}

        def emit_front(c):
            po0 = psum.tile([P, S], f32, tag="psO", bufs=5,
                            name=f"ps_o{c}_0")
            nc.tensor.matmul(po0, lhsT=ind_t[:, c * P:(c + 1) * P],
                             rhs=lewb, start=True, stop=False)
            po0s[c] = po0
            pst = []
            for mg in range(KH):
                p_ = psum.tile([P, S], f32, tag="psT", bufs=3,
                               name=f"ps_t{c}_{mg}")
                off = c * H + mg * P
                for kh in range(KH):
                    nc.tensor.matmul(p_, lhsT=usb[kh][:, off:off + P],
                                     rhs=hst[kh], start=(kh == 0),
                                     stop=(kh == KH - 1))
                pst.append(p_)
            psts[c] = pst

        emit_front(0)
        for c in range(C):
            po0 = po0s.pop(c)
            bc = bcp.tile([P, S], f32, tag="bc", bufs=2, name=f"bc{c}")
            nc.scalar.activation(bc, po0, CP)
            tcb = tcp.tile([P, KH * S], bf16, tag="tc", bufs=2,
                           name=f"tc{c}")
            for mg in range(KH):
                nc.scalar.activation(tcb[:, mg * S:(mg + 1) * S],
                                     psts[c][mg], CP)
            del psts[c]
            if c + 1 < C:
                emit_front(c + 1)

            ot = outp.tile([P, NS * S], f32, tag="ot", bufs=4,
                           name=f"ot{c}")
            for st in range(1, NS):
                po = psum.tile([P, S], f32, tag="psO", bufs=5,
                               name=f"ps_o{c}_{st}")
                for kg in range(KH):
                    nc.tensor.matmul(po,
                                     lhsT=tcb[:, kg * S + st * P:
                                              kg * S + (st + 1) * P],
                                     rhs=het[kg], start=(kg == 0),
                                     stop=(kg == KH - 1))
                nc.vector.scalar_tensor_tensor(ot[:, st * S:(st + 1) * S],
                                               po, ls_t[st][:, c:c + 1], bc,
                                               op0=ADD, op1=ADD)
            # st 0 last: accumulate biaff onto the broadcast row already in
            # the bank (after ACT's snapshot read).
            for kg in range(KH):
                nc.tensor.matmul(po0, lhsT=tcb[:, kg * S:kg * S + P],
                                 rhs=het[kg], start=False,
                                 stop=(kg == KH - 1))
            nc.scalar.activation(ot[:, 0:S], po0, ID,
                                 bias=ls_t[0][:, c:c + 1])
            eng = nc.sync if c % 2 else nc.gpsimd
            dst = out_d[c].rearrange("(st p) e -> p st e", p=P)
            eng.dma_start(dst, ot.rearrange("p (st e) -> p st e", e=S))

    nc.compile()
    return nc


def _host_prep(seq_feats, U, W_w, W_b, Ws_w, Ws_b, We_w, We_b):
    """Build the per-core input maps (host-side layout prep, all small
    except seq_feats transpose)."""
    ind = np.zeros((C, C * P), dtype=BF16)
    for k in range(C):
        ind[k, k * P:(k + 1) * P] = 1.0

    pin_w = np.concatenate([Ws_w.T, We_w.T], axis=1).astype(BF16)  # [D, 2H]
    common = {
        "u": np.ascontiguousarray(U.reshape(H, C * H)).astype(BF16),
        "wsb": np.ascontiguousarray(Ws_b.reshape(KH, P, 1)).astype(np.float32),
        "web": np.ascontiguousarray(We_b.reshape(KH, P, 1)).astype(np.float32),
        "wstl": np.ascontiguousarray(W_w[:, :H].T).astype(BF16),
        "wetl": np.ascontiguousarray(W_w[:, H:].T).astype(BF16),
        "wb": np.ascontiguousarray(W_b.reshape(C, 1)).astype(np.float32),
        "ind": ind,
    }
    in_maps = []
    for b in range(B):
        m = dict(common)
        pin = np.empty((D, S + 2 * H), dtype=BF16)
        pin[:, :S] = seq_feats[b].T
        pin[:, S:] = pin_w
        m["pin"] = pin
        in_maps.append(m)
    return in_maps


def kernel(seq_feats, U, W_w, W_b, Ws_w, Ws_b, We_w, We_b):
    from concourse.bass_utils import run_bass_kernel_spmd

    seq_feats = np.asarray(seq_feats, dtype=np.float32)
    U = np.asarray(U, dtype=np.float32)
    W_w = np.asarray(W_w, dtype=np.float32)
    W_b = np.asarray(W_b, dtype=np.float32)
    Ws_w = np.asarray(Ws_w, dtype=np.float32)
    Ws_b = np.asarray(Ws_b, dtype=np.float32)
    We_w = np.asarray(We_w, dtype=np.float32)
    We_b = np.asarray(We_b, dtype=np.float32)

    if "nc" not in _CACHE:
        _CACHE["nc"] = _build_nc()
    nc = _CACHE["nc"]

    in_maps = _host_prep(seq_feats, U, W_w, W_b, Ws_w, Ws_b, We_w, We_b)
    res = run_bass_kernel_spmd(nc, in_maps, core_ids=list(range(N_CORES)))
    _CACHE["last_result"] = res

    out = np.stack([res.results[b]["out"] for b in range(B)])  # [B, C, S, S]
    return np.ascontiguousarray(out.transpose(0, 2, 3, 1)).astype(np.float32)
